# revision 4
# baseline (speedup 1.0000x reference)
"""GNN message passing (30x GCNConv + 8x GatedGraphConv/GRU) on 8 trn2 cores.

Strategy: partition nodes (dst) across 8 cores. Each layer:
  dense transform (feature-major, PE) -> transpose to node-major ->
  AllGather the transformed table -> SpMM via dma_gather (256B rows) +
  dma_scatter_add into bank-replicated accumulators (4 banks so that any
  single scatter call has unique target rows; calls serialize via WAW) ->
  merge banks -> transpose back -> pointwise.

GCN norm factorizes: norm = dinv[src]*dinv[dst], so
  h' = dinv (.) (A @ (dinv (.) (h W^T)) + dinv (.) (h W^T)) + b
needs only per-node scalings, no per-edge weights.
"""
import sys
if "/opt/trn_rl_repo" not in sys.path:
    sys.path.insert(0, "/opt/trn_rl_repo")

import numpy as np

N, E, D = 50000, 800000, 64
N_GCN, N_GGC = 30, 8
NCORES = 8
NSH = N // NCORES            # 6250 real nodes per core
NT = (NSH + 127) // 128      # 49 tiles
NSHP = NT * 128              # 6272 padded
NFULL = NCORES * NSHP        # 50176 padded table rows
LO_LIM = 32768
HI_BASE = NFULL - 32768      # 17408
BANKS = 4
SBK = NSHP + 384             # 6656 rows per accumulator bank
NACCR = BANKS * SBK          # 26624
S_MAX = 4096                 # max idxs per SWDGE call
SINGLE_PACKET = False
SCRATCH = 49152             # SWDGE descriptor-ring carveout bytes

_CACHE = {}


# ----------------------------------------------------------------- host prep
def _pos(idx):
    return (idx // NSH) * NSHP + (idx % NSH)


def _build_edge_tables(src, dst):
    """Per-core gather/scatter int16 index streams + call size lists."""
    src = np.asarray(src, np.int64)
    dst = np.asarray(dst, np.int64)
    deg = np.bincount(dst, minlength=N).astype(np.int64) + 1
    dinv = deg.astype(np.float64) ** -0.5

    core = dst // NSH
    dst_local = dst - core * NSH
    srcp = _pos(src)
    lo = srcp < LO_LIM

    # bank = per-dst occurrence index mod BANKS
    order = np.argsort(dst, kind="stable")
    rank = np.empty(E, np.int64)
    rank[order] = np.arange(E) - np.repeat(
        np.concatenate([[0], np.cumsum(np.bincount(dst, minlength=N))[:-1]]),
        np.bincount(dst, minlength=N))
    bank = rank % BANKS
    srow = bank * SBK + dst_local

    # call counts per segment
    def seg_calls(mask):
        # group key = (core, srow); call = (rank_in_group + srow) % C
        ncalls_size = 0
        max_group = 1
        cnts = np.zeros(NCORES, np.int64)
        for c in range(NCORES):
            m = mask & (core == c)
            cnts[c] = m.sum()
            if cnts[c]:
                _, gc = np.unique(srow[m], return_counts=True)
                max_group = max(max_group, int(gc.max()))
        C = max(int(np.ceil(cnts.max() / S_MAX)), max_group)
        if SINGLE_PACKET:
            C = max(C, int(np.ceil(cnts.max() / 1024)))
        return C

    C_lo = seg_calls(lo)
    C_hi = seg_calls(~lo)

    # assign each edge to a call
    def assign(mask, C):
        per = []  # per core: list of C lists of (gidx, srow)
        for c in range(NCORES):
            m = np.where(mask & (core == c))[0]
            sr = srow[m]
            o = np.argsort(sr, kind="stable")
            m = m[o]; sr = sr[o]
            grp_start = np.concatenate([[True], sr[1:] != sr[:-1]])
            grp_id = np.cumsum(grp_start) - 1
            starts = np.where(grp_start)[0]
            rank_in = np.arange(len(m)) - starts[grp_id]
            call = (rank_in + sr) % C
            calls = []
            for k in range(C):
                sel = m[call == k]
                g = srcp[sel]
                s = srow[sel]
                o2 = np.argsort(g, kind="stable")
                calls.append((g[o2], s[o2]))
            per.append(calls)
        return per

    lo_per = assign(lo, C_lo)
    hi_per = assign(~lo, C_hi)

    # pad calls to uniform sizes across cores
    def finalize(per, C, hi_seg):
        sizes = []
        for k in range(C):
            mx = max(len(per[c][k][0]) for c in range(NCORES))
            sizes.append(int(np.ceil(max(mx, 16) / 16) * 16))
        gstreams, sstreams = [], []
        for c in range(NCORES):
            gs, ss = [], []
            for k in range(C):
                g, s = per[c][k]
                npad = sizes[k] - len(g)
                if npad:
                    j = np.arange(npad)
                    drow = (j % BANKS) * SBK + NSHP + (j // BANKS) % 384
                    assert npad <= BANKS * 384
                    g = np.concatenate([g, np.zeros(npad, np.int64)
                                        + (HI_BASE if hi_seg else 0)])
                    s = np.concatenate([s, drow])
                if hi_seg:
                    g = g - HI_BASE
                assert g.min() >= 0 and g.max() < 32768, (g.min(), g.max())
                assert s.max() < 32768
                gs.append(g.astype(np.int16))
                ss.append(s.astype(np.int16))
            gstreams.append(np.concatenate(gs))
            sstreams.append(np.concatenate(ss))
        return sizes, gstreams, sstreams

    LO_SIZES, glo, slo = finalize(lo_per, C_lo, False)
    HI_SIZES, ghi, shi = finalize(hi_per, C_hi, True)

    gidx = [np.concatenate([glo[c], ghi[c]]) for c in range(NCORES)]
    sidx = [np.concatenate([slo[c], shi[c]]) for c in range(NCORES)]

    def wrap(a):
        return np.tile(a.reshape(-1, 16).T, (8, 1)).copy()

    gidx = [wrap(a) for a in gidx]
    sidx = [wrap(a) for a in sidx]
    return LO_SIZES, HI_SIZES, gidx, sidx, dinv


# ------------------------------------------------------------- kernel build
def _build_program(LO_SIZES, HI_SIZES):
    import concourse.bacc as bacc
    import concourse.tile as tile
    import concourse.mybir as mybir
    from concourse.bass import MemorySpace

    f32 = mybir.dt.float32
    i16 = mybir.dt.int16
    TOT = sum(LO_SIZES) + sum(HI_SIZES)

    nc = bacc.Bacc("TRN2", target_bir_lowering=False, debug=False,
                   num_devices=NCORES, dynamic_dma_scratch_size=SCRATCH)

    # inputs (per core)
    xT_d = nc.dram_tensor("xT", [128, NSHP], f32, kind="ExternalInput")
    linWT_d = nc.dram_tensor("linWT", [128, 64], f32, kind="ExternalInput")
    gcnWT_d = nc.dram_tensor("gcnWT", [64, N_GCN * 64], f32, kind="ExternalInput")
    gcnB_d = nc.dram_tensor("gcnB", [64, N_GCN + 1], f32, kind="ExternalInput")
    ggcW_d = nc.dram_tensor("ggcW", [64, N_GGC * 64], f32, kind="ExternalInput")
    wihT_d = nc.dram_tensor("wihT", [64, 192], f32, kind="ExternalInput")
    whhT_d = nc.dram_tensor("whhT", [64, 192], f32, kind="ExternalInput")
    gruB_d = nc.dram_tensor("gruB", [64, 4], f32, kind="ExternalInput")
    dnm_d = nc.dram_tensor("dinv_nm", [128, NT * 64], f32, kind="ExternalInput")
    eye_d = nc.dram_tensor("eye", [128, 128], f32, kind="ExternalInput")
    gidx_d = nc.dram_tensor("gidx", [128, TOT // 16], i16, kind="ExternalInput")
    sidx_d = nc.dram_tensor("sidx", [128, TOT // 16], i16, kind="ExternalInput")

    hout_d = nc.dram_tensor("hout", [NSHP, 64], f32, kind="ExternalOutput")

    u_loc = nc.dram_tensor("u_loc", [NSHP, 64], f32)
    u_full = nc.dram_tensor("u_full", [NFULL, 64], f32, addr_space="Shared")
    u_work = nc.dram_tensor("u_work", [NFULL, 64], f32)
    accum = nc.dram_tensor("accum", [NACCR, 64], f32)

    CHUNKS = []  # free-dim chunks of 512 over NSHP
    off = 0
    while off < NSHP:
        w = min(512, NSHP - off)
        CHUNKS.append((off, w))
        off += w

    TGROUPS = []  # groups of <=4 tiles for transposes
    t = 0
    while t < NT:
        g = min(4, NT - t)
        TGROUPS.append((t, g))
        t += g

    with tile.TileContext(nc) as tc:
        with (
            tc.tile_pool(name="const", bufs=1) as cp,
            tc.tile_pool(name="hbuf", bufs=1) as hp,
            tc.tile_pool(name="nbuf", bufs=1) as nbp,
            tc.tile_pool(name="msg", bufs=2) as mp,
            tc.tile_pool(name="idx", bufs=2) as ixp,
            tc.tile_pool(name="gru", bufs=2) as gp,
            tc.tile_pool(name="psA", bufs=4, space="PSUM") as psA,
            tc.tile_pool(name="psB", bufs=4, space="PSUM") as psB,
        ):
            # persistent SBUF state
            linWT = cp.tile([128, 64], f32)
            gcnWT = cp.tile([64, N_GCN * 64], f32)
            gcnB = cp.tile([64, N_GCN + 1], f32)
            ggcW = cp.tile([64, N_GGC * 64], f32)
            wihT = cp.tile([64, 192], f32)
            whhT = cp.tile([64, 192], f32)
            gruB = cp.tile([64, 4], f32)
            dnm = cp.tile([128, NT * 64], f32)
            eye = cp.tile([128, 128], f32)
            zeros = cp.tile([128, 28 * 64], f32)
            hT = hp.tile([64, NSHP], f32, tag="hT")
            vT = hp.tile([64, NSHP], f32, tag="vT")
            u_buf = nbp.tile([128, NT, 64], f32, tag="ubuf")
            s_buf = nbp.tile([128, NT, 64], f32, tag="sbuf")

            for tl, dr in [(linWT, linWT_d), (gcnWT, gcnWT_d), (gcnB, gcnB_d),
                           (ggcW, ggcW_d), (wihT, wihT_d), (whhT, whhT_d),
                           (gruB, gruB_d), (dnm, dnm_d),
                           (eye, eye_d)]:
                nc.sync.dma_start(out=tl[:], in_=dr[:])
            nc.vector.memset(zeros[:], 0.0)

            # ---- initial dense: hT = relu(linW @ xT + lb) (lb folded in x? no bias ap)
            # linear_b handled via gcnB_d last column
            linB = gcnB[:, N_GCN:N_GCN + 1]
            for off, w in CHUNKS:
                xc = ixp.tile([128, 512], f32, tag="xc")
                nc.sync.dma_start(out=xc[:, :w], in_=xT_d[:, off:off + w])
                ps = psA.tile([64, 512], f32, tag="psA")
                nc.tensor.matmul(ps[:, :w], linWT[:], xc[:, :w])
                nc.scalar.activation(hT[:, off:off + w], ps[:, :w],
                                     mybir.ActivationFunctionType.Relu,
                                     bias=linB, scale=1.0)

            def fm_to_nm(srcT, dst_buf, scale_bcast):
                """dst_buf[128, NT, 64] = transpose of srcT [64, NSHP];
                optionally multiplied by scale_bcast afterwards (fat DVE)."""
                for t0, g in TGROUPS:
                    ps = psB.tile([128, 256], f32, tag="psB")
                    for k in range(g):
                        tt = t0 + k
                        nc.tensor.matmul(
                            ps[:, k * 64:(k + 1) * 64],
                            srcT[:, tt * 128:(tt + 1) * 128],
                            eye[:64, :64])
                    nc.scalar.activation(
                        dst_buf[:, t0:t0 + g, :].rearrange("p t f -> p (t f)"),
                        ps[:, :g * 64],
                        mybir.ActivationFunctionType.Copy)
                if scale_bcast is not None:
                    flat = dst_buf[:].rearrange("p t f -> p (t f)")
                    nc.vector.tensor_mul(flat, flat, scale_bcast[:])

            def nm_to_fm(src_buf, dstT):
                for t0, g in TGROUPS:
                    ps = psB.tile([64, 512], f32, tag="psB")
                    for k in range(g):
                        tt = t0 + k
                        nc.tensor.matmul(
                            ps[:, k * 128:(k + 1) * 128],
                            src_buf[:, tt, :], eye[:])
                    nc.scalar.activation(
                        dstT[:, t0 * 128:(t0 + g) * 128], ps[:, :g * 128],
                        mybir.ActivationFunctionType.Copy)

            def spmm():
                """u_loc -> allgather -> gather/scatter -> merged into s_buf."""
                nc.gpsimd.collective_compute(
                    "AllGather", mybir.AluOpType.bypass,
                    replica_groups=[list(range(NCORES))],
                    ins=[u_loc[:]], outs=[u_full[:]])
                # offset-slice reads of a collective output hang; bounce to Local
                nc.sync.dma_start(out=u_work[:], in_=u_full[:])
                for b in range(BANKS):
                    for r0 in (0, NSHP - 28 * 128):
                        nc.sync.dma_start(
                            out=accum.ap()[b * SBK + r0:b * SBK + r0 + 28 * 128, :]
                                .rearrange("(t p) f -> p t f", p=128),
                            in_=zeros[:])
                ioff = 0
                for hi_seg, sizes in ((0, LO_SIZES), (1, HI_SIZES)):
                    table = (u_work.ap()[HI_BASE:NFULL, :] if hi_seg
                             else u_work.ap()[0:LO_LIM, :])
                    for S in sizes:
                        gtl = ixp.tile([128, S // 16], i16, tag="gi")
                        stl = ixp.tile([128, S // 16], i16, tag="si")
                        nc.sync.dma_start(out=gtl[:],
                                          in_=gidx_d[:, ioff:ioff + S // 16])
                        nc.sync.dma_start(out=stl[:],
                                          in_=sidx_d[:, ioff:ioff + S // 16])
                        msg = mp.tile([128, (S + 127) // 128, 64], f32, tag="msg")
                        nc.gpsimd.dma_gather(
                            msg[:, :(S + 127) // 128, :], table, gtl[:], S, S, 64,
                            single_packet=SINGLE_PACKET)
                        nc.gpsimd.dma_scatter_add(
                            accum[:], msg[:, :(S + 127) // 128, :], stl[:], S, S, 64,
                            single_packet=SINGLE_PACKET)
                        ioff += S // 16
                # readback + merge banks (CCE add on the SBUF write side)
                nc.sync.dma_start(
                    out=s_buf[:],
                    in_=accum.ap()[0:NSHP, :].rearrange("(t p) f -> p t f", p=128))
                import concourse.mybir as _mb
                for b in range(1, BANKS):
                    nc.gpsimd.dma_start(
                        out=s_buf[:],
                        in_=accum.ap()[b * SBK:b * SBK + NSHP, :]
                            .rearrange("(t p) f -> p t f", p=128),
                        accum_op=_mb.AluOpType.add)

            # ---------------- 30 GCN layers ----------------
            for i in range(N_GCN):
                # vT = (h @ W_i^T)^T
                for off, w in CHUNKS:
                    ps = psA.tile([64, 512], f32, tag="psA")
                    nc.tensor.matmul(ps[:, :w], gcnWT[:, i * 64:(i + 1) * 64],
                                     hT[:, off:off + w])
                    nc.scalar.activation(vT[:, off:off + w], ps[:, :w],
                                         mybir.ActivationFunctionType.Copy)
                # u = dinv (.) v  (node-major)
                fm_to_nm(vT, u_buf, dnm)
                nc.sync.dma_start(
                    out=u_loc.ap().rearrange("(t p) f -> p t f", p=128),
                    in_=u_buf[:])
                spmm()
                # s2 = A@u + u ; hT = dinv (.) s2^T + b
                sflat = s_buf[:].rearrange("p t f -> p (t f)")
                nc.vector.tensor_add(sflat, sflat, u_buf[:].rearrange("p t f -> p (t f)"))
                nc.vector.tensor_mul(sflat, sflat, dnm[:])
                nm_to_fm(s_buf, hT)
                nc.vector.tensor_scalar_add(hT[:], hT[:], gcnB[:, i:i + 1])

            # ---------------- 8 GGC/GRU layers ----------------
            for j in range(N_GGC):
                for off, w in CHUNKS:
                    ps = psA.tile([64, 512], f32, tag="psA")
                    nc.tensor.matmul(ps[:, :w], ggcW[:, j * 64:(j + 1) * 64],
                                     hT[:, off:off + w])
                    nc.scalar.activation(vT[:, off:off + w], ps[:, :w],
                                         mybir.ActivationFunctionType.Copy)
                fm_to_nm(vT, u_buf, None)
                nc.sync.dma_start(
                    out=u_loc.ap().rearrange("(t p) f -> p t f", p=128),
                    in_=u_buf[:])
                spmm()
                # mT = s^T (feature-major aggregated messages)
                nm_to_fm(s_buf, vT)
                mT = vT
                Sig = mybir.ActivationFunctionType.Sigmoid
                for off, w in CHUNKS:
                    hc = hT[:, off:off + w]
                    mc = mT[:, off:off + w]
                    ps_r = psA.tile([64, 512], f32, tag="psA")
                    nc.tensor.matmul(ps_r[:, :w], wihT[:, 0:64], mc,
                                     start=True, stop=False)
                    nc.tensor.matmul(ps_r[:, :w], whhT[:, 0:64], hc,
                                     start=False, stop=True)
                    r = gp.tile([64, 512], f32, tag="r")
                    nc.scalar.activation(r[:, :w], ps_r[:, :w], Sig,
                                         bias=gruB[:, 0:1])
                    ps_z = psA.tile([64, 512], f32, tag="psA")
                    nc.tensor.matmul(ps_z[:, :w], wihT[:, 64:128], mc,
                                     start=True, stop=False)
                    nc.tensor.matmul(ps_z[:, :w], whhT[:, 64:128], hc,
                                     start=False, stop=True)
                    z = gp.tile([64, 512], f32, tag="z")
                    nc.scalar.activation(z[:, :w], ps_z[:, :w], Sig,
                                         bias=gruB[:, 1:2])
                    ps_n = psA.tile([64, 512], f32, tag="psA")
                    nc.tensor.matmul(ps_n[:, :w], wihT[:, 128:192], mc)
                    ps_h = psA.tile([64, 512], f32, tag="psA")
                    nc.tensor.matmul(ps_h[:, :w], whhT[:, 128:192], hc)
                    hn = gp.tile([64, 512], f32, tag="hn")
                    nc.scalar.activation(hn[:, :w], ps_h[:, :w],
                                         mybir.ActivationFunctionType.Identity,
                                         bias=gruB[:, 3:4])
                    nc.vector.tensor_mul(r[:, :w], r[:, :w], hn[:, :w])
                    nc.vector.tensor_add(r[:, :w], r[:, :w], ps_n[:, :w])
                    nc.scalar.activation(hn[:, :w], r[:, :w],
                                         mybir.ActivationFunctionType.Tanh,
                                         bias=gruB[:, 2:3])
                    nc.vector.tensor_sub(r[:, :w], hc, hn[:, :w])
                    nc.vector.tensor_mul(r[:, :w], r[:, :w], z[:, :w])
                    nc.vector.tensor_add(hc, r[:, :w], hn[:, :w])

            # ---------------- output ----------------
            fm_to_nm(hT, u_buf, None)
            nc.sync.dma_start(
                out=hout_d.ap().rearrange("(t p) f -> p t f", p=128),
                in_=u_buf[:])
    nc.compile()
    return nc


# ----------------------------------------------------------------- runner
class _Runner:
    def __init__(self, nc, n_cores):
        import jax
        import concourse.mybir as mybir
        from concourse import bass2jax
        from concourse.bass2jax import _bass_exec_p, install_neuronx_cc_hook
        from jax.sharding import Mesh, PartitionSpec
        from jax.experimental.shard_map import shard_map
        install_neuronx_cc_hook()
        self.jax = jax
        self.n_cores = n_cores
        pname = nc.partition_id_tensor.name if nc.partition_id_tensor else None
        in_names, out_names, out_avals, zero_outs = [], [], [], []
        for alloc in nc.m.functions[0].allocations:
            if not isinstance(alloc, mybir.MemoryLocationSet):
                continue
            name = alloc.memorylocations[0].name
            if alloc.kind == "ExternalInput":
                if name != pname:
                    in_names.append(name)
            elif alloc.kind == "ExternalOutput":
                shape = tuple(alloc.tensor_shape)
                dtype = mybir.dt.np(alloc.dtype)
                out_names.append(name)
                out_avals.append(jax.core.ShapedArray(shape, dtype))
                zero_outs.append(np.zeros(shape, dtype))
        self.in_names, self.out_names = in_names, out_names
        self.out_avals, self.zero_outs = out_avals, zero_outs
        n_params, n_outs = len(in_names), len(out_avals)
        all_in = in_names + out_names + ([pname] if pname else [])
        donate = tuple(range(n_params, n_params + n_outs))

        def _body(*args):
            operands = list(args)
            if pname is not None:
                operands.append(bass2jax.partition_id_tensor())
            return tuple(_bass_exec_p.bind(
                *operands, out_avals=tuple(out_avals), in_names=tuple(all_in),
                out_names=tuple(out_names), lowering_input_output_aliases=(),
                sim_require_finite=True, sim_require_nnan=True, nc=nc))

        devices = jax.devices()[:n_cores]
        mesh = Mesh(np.asarray(devices), ("core",))
        self._fn = jax.jit(
            shard_map(_body, mesh=mesh,
                      in_specs=(PartitionSpec("core"),) * (n_params + n_outs),
                      out_specs=(PartitionSpec("core"),) * n_outs,
                      check_rep=False),
            donate_argnums=donate, keep_unused=True)
        self._n_params = n_params

    def prep(self, in_maps):
        per = [[np.asarray(m[n]) for n in self.in_names] for m in in_maps]
        return [np.concatenate([per[c][i] for c in range(self.n_cores)], 0)
                for i in range(self._n_params)]

    def run(self, concat_in):
        zz = [np.zeros((self.n_cores * z.shape[0], *z.shape[1:]), z.dtype)
              for z in self.zero_outs]
        out = self._fn(*concat_in, *zz)
        self.jax.block_until_ready(out)
        return [
            {n: np.asarray(out[i]).reshape(self.n_cores, *self.out_avals[i].shape)[c]
             for i, n in enumerate(self.out_names)}
            for c in range(self.n_cores)
        ]


def _get_runner(inputs):
    key = (inputs["edge_index"].tobytes()[:1024], int(inputs["edge_index"].sum()))
    if key in _CACHE:
        return _CACHE[key]
    src = inputs["edge_index"][0].astype(np.int64)
    dst = inputs["edge_index"][1].astype(np.int64)
    LO_SIZES, HI_SIZES, gidx, sidx, dinv = _build_edge_tables(src, dst)
    nc = _build_program(LO_SIZES, HI_SIZES)
    runner = _Runner(nc, NCORES)
    _CACHE[key] = (runner, gidx, sidx, dinv)
    return _CACHE[key]


def _in_maps(inputs, gidx, sidx, dinv):
    x = np.asarray(inputs["x"], np.float32)
    linW = np.asarray(inputs["linear_w"], np.float32)
    linb = np.asarray(inputs["linear_b"], np.float32)
    gcw = np.asarray(inputs["gcn_w"], np.float32)
    gcb = np.asarray(inputs["gcn_b"], np.float32)
    ggw = np.asarray(inputs["ggc_w"], np.float32)
    wih = np.asarray(inputs["gru_w_ih"], np.float32)
    whh = np.asarray(inputs["gru_w_hh"], np.float32)
    bih = np.asarray(inputs["gru_b_ih"], np.float32)
    bhh = np.asarray(inputs["gru_b_hh"], np.float32)

    gcnWT = np.concatenate([gcw[i].T for i in range(N_GCN)], 1)  # [64, 30*64]
    gcnB = np.concatenate([gcb.T, linb[:, None]], 1)             # [64, 31]
    ggcW = np.concatenate([ggw[j] for j in range(N_GGC)], 1)     # [64, 8*64]
    wihT = np.concatenate([wih[g * 64:(g + 1) * 64].T for g in range(3)], 1)
    whhT = np.concatenate([whh[g * 64:(g + 1) * 64].T for g in range(3)], 1)
    gruB = np.stack([bih[0:64] + bhh[0:64], bih[64:128] + bhh[64:128],
                     bih[128:192], bhh[128:192]], 1)             # [64,4]
    eye = np.eye(128, dtype=np.float32)

    maps = []
    for c in range(NCORES):
        xs = np.zeros((NSHP, 128), np.float32)
        xs[:NSH] = x[c * NSH:(c + 1) * NSH]
        dv = np.zeros(NSHP, np.float32)
        dv[:NSH] = dinv[c * NSH:(c + 1) * NSH].astype(np.float32)
        dnm = np.repeat(
            dv.reshape(NT, 128).T[:, :, None], 64, axis=2).reshape(128, NT * 64)
        maps.append({
            "xT": np.ascontiguousarray(xs.T),
            "linWT": np.ascontiguousarray(linW.T),
            "gcnWT": np.ascontiguousarray(gcnWT),
            "gcnB": np.ascontiguousarray(gcnB),
            "ggcW": np.ascontiguousarray(ggcW),
            "wihT": np.ascontiguousarray(wihT),
            "whhT": np.ascontiguousarray(whhT),
            "gruB": np.ascontiguousarray(gruB),
            "dinv_nm": np.ascontiguousarray(dnm),
            "eye": eye,
            "gidx": gidx[c],
            "sidx": sidx[c],
        })
    return maps


def kernel(**inputs):
    runner, gidx, sidx, dinv = _get_runner(inputs)
    maps = _in_maps(inputs, gidx, sidx, dinv)
    ci = runner.prep(maps)
    last_err = None
    for attempt in range(5):
        try:
            res = runner.run(ci)
            break
        except Exception as e:  # transient axon INTERNAL errors
            last_err = e
    else:
        raise last_err
    return np.concatenate([res[c]["hout"][:NSH] for c in range(NCORES)], 0)
